# revision 35
# baseline (speedup 1.0000x reference)
"""MoE ConditionalFeedForward kernel for 8 trn2 NeuronCores.

Strategy: expert parallelism with 2-way intermediate (I) splitting for load
balance. The 8 experts are ranked by routed-token count and paired
heaviest-with-lightest into 4 groups; the two cores of group g each own HALF
the I-rows (22 of 44 128-row blocks) of BOTH experts in the group and process
ALL tokens routed to them. Each core therefore streams exactly 44 block-rows
of w1/w3/w2 (the same HBM traffic as one full expert) while its matmul column
count is bounded by max-heavy + max-light counts instead of the global max.
Token (t, slot) pairs whose two slots hit the same expert are deduplicated
(computed once, scattered twice).

Per core, slice s in {heavy, light} with capacity C_s:
  phase 1: hT[i, c] = silu(sum_d w1T[d,i] xT[d,c]) * (sum_d w3T[d,i] xT[d,c])
           for the 22 owned i-blocks (PE matmuls, d on partitions)
  phase 2: yT[d, c] = sum_{i in owned} hT[i, c] * w2[i, d]  (partial sum)
The two partial yT of an expert are summed on the host (f32) and scattered.

All weights/activations stream as bf16; PSUM accumulation is f32.
"""

import numpy as np
import ml_dtypes

BF16 = ml_dtypes.bfloat16

# Problem dims (hardcoded per contract; kernel.py must be self-contained).
T, A, E, D, I = 1024, 2, 8, 2048, 5632
N_CORES = 8
QB = 8                 # startup i-blocks of slice-a w1/w3 sent as fp8
DB = D // 128          # 16 d-chunks (phase-1 contraction)
IB = I // 128          # 44 i-blocks total per expert
HB = IB // 2           # 22 i-blocks per core slice
NPASS = 8              # phase-2 passes over D
W = D // NPASS         # 256 output columns per phase-2 pass
NDC = W // 128         # 2 128-col d-blocks per pass
GSZ = 8                # w2 i-blocks per DMA group

_BUILD_CACHE = {}


def _pad4(n):
    return max(4, -(-int(n) // 4) * 4)


def _build(C1, C2):
    """Build + compile the per-core Bass program for slice capacities C1, C2."""
    key = (C1, C2)
    if key in _BUILD_CACHE:
        return _BUILD_CACHE[key]

    import concourse.mybir as mybir
    import concourse.tile as tile
    from concourse import bacc

    dt = mybir.dt
    WDT = dt.bfloat16
    F8E3 = dt.float8e3
    F32 = dt.float32

    assert C1 <= 512 and C2 <= C1

    nc = bacc.Bacc("TRN2", target_bir_lowering=False, debug=False,
                   num_devices=N_CORES)

    xa_t = nc.dram_tensor("xga", [128, DB * C1], WDT, kind="ExternalInput").ap()
    xb_t = nc.dram_tensor("xgb", [128, DB * C2], WDT, kind="ExternalInput").ap()
    # The first QB i-blocks of slice-a's w1/w3 stream as fp8 e3m4: during the
    # DMA ramp-up window (~first 15us) the weight stream can't keep ahead of
    # warm compute in bf16; halving the startup bytes removes the early
    # stalls that used to re-throttle the PE clock. Costs +0.58% rel err.
    w1aq_t = nc.dram_tensor("w1aq", [128, QB * DB * 128], F8E3,
                            kind="ExternalInput").ap()
    w3aq_t = nc.dram_tensor("w3aq", [128, QB * DB * 128], F8E3,
                            kind="ExternalInput").ap()
    w1a_t = nc.dram_tensor("w1a", [128, (HB - QB) * DB * 128], WDT,
                           kind="ExternalInput").ap()
    w3a_t = nc.dram_tensor("w3a", [128, (HB - QB) * DB * 128], WDT,
                           kind="ExternalInput").ap()
    w1b_t = nc.dram_tensor("w1b", [128, HB * DB * 128], WDT,
                           kind="ExternalInput").ap()
    w3b_t = nc.dram_tensor("w3b", [128, HB * DB * 128], WDT,
                           kind="ExternalInput").ap()
    # w2 streams as fp8 e3m4 (pre-scaled by 2**7 on host; descale fused into
    # the PSUM drain). The PE takes fp8e3 stationary x bf16 moving at full
    # rate, so this halves w2's HBM bytes at no PE cost; measured end-to-end
    # rel err 1.39% vs the 2e-2 gate.
    w2a_t = nc.dram_tensor("w2a", [NPASS, 128, HB * W], F8E3,
                           kind="ExternalInput").ap()
    w2b_t = nc.dram_tensor("w2b", [NPASS, 128, HB * W], F8E3,
                           kind="ExternalInput").ap()
    # outputs are y transposed ([D, C]) partial sums; host adds + untransposes.
    # bf16 partials (summed as f32 on host) halve the output DMA traffic,
    # which matters in the phase-2 windows where w2 streaming + y drain
    # together are at the DMA roofline.
    ya_t = nc.dram_tensor("yta", [D, C1], WDT, kind="ExternalOutput").ap()
    yb_t = nc.dram_tensor("ytb", [D, C2], WDT, kind="ExternalOutput").ap()

    slices = [(C1, xa_t, w1a_t, w3a_t, w2a_t, ya_t),
              (C2, xb_t, w1b_t, w3b_t, w2b_t, yb_t)]
    wq_t = {0: (w1aq_t, w3aq_t)}

    with tile.TileContext(nc) as tc:
        with (
            tc.tile_pool(name="xpool", bufs=1) as xpool,
            tc.tile_pool(name="w1pool", bufs=5) as w1pool,
            tc.tile_pool(name="w3pool", bufs=5) as w3pool,
            tc.tile_pool(name="wq1pool", bufs=3) as wq1pool,
            tc.tile_pool(name="wq3pool", bufs=3) as wq3pool,
            # deep w2 prefetch: fills the DMA-idle trough at each phase-1
            # tail (w1/w3 fully delivered ~20us before phase 1 ends) so
            # phase 2 never starves at pass boundaries.
            tc.tile_pool(name="w2pool", bufs=20) as w2pool,
            tc.tile_pool(name="hpool", bufs=1) as hpool,
            tc.tile_pool(name="spool", bufs=2) as spool,
            tc.tile_pool(name="opool", bufs=4) as opool,
            tc.tile_pool(name="ps", bufs=2, space="PSUM") as ps,
        ):
            xg = {}
            for s, (C, x_t, *_r) in enumerate(slices):
                xg[s] = xpool.tile([128, DB * C], WDT, tag=f"x{s}",
                                   name=f"xg{s}")

            # HAM warmup: dummy matmuls with no DMA dependency keep the PE
            # busy from t=0 so the 4096-cycle activity window fires and the
            # clock is at 2.4GHz by the time real data lands (~2.5us). They
            # also fill the startup DMA-wait that used to idle the PE.
            wu = xpool.tile([128, 256], WDT, tag="wu", name="wu")
            nc.vector.memset(wu[:, :], 0.0)
            wups = ps.tile([128, 256], F32, tag="y0", name="wups")
            for _ in range(13):
                nc.tensor.matmul(wups[:, :], wu[:, :128], wu[:, :256],
                                 start=True, stop=True)

            def issue_w13(s, gi, nb, b0):
                """Create + DMA one phase-1 weight group for slice s.
                Slice-a blocks < QB read the fp8 dram copies."""
                qt = s == 0 and b0 < QB
                if qt:
                    w1_t, w3_t = wq_t[0]
                    wt1 = wq1pool.tile([128, 2 * DB * 128], F8E3, tag="w1q",
                                       name="wt1q")
                    wt3 = wq3pool.tile([128, 2 * DB * 128], F8E3, tag="w3q",
                                       name="wt3q")
                    lo = b0 * DB * 128
                else:
                    w1_t, w3_t = slices[s][2], slices[s][3]
                    wt1 = w1pool.tile([128, 2 * DB * 128], WDT, tag="w1",
                                      name="wt1")
                    wt3 = w3pool.tile([128, 2 * DB * 128], WDT, tag="w3",
                                      name="wt3")
                    lo = (b0 - (QB if s == 0 else 0)) * DB * 128
                if s == 0 and gi == 0:
                    # startup-critical: interleave x / w1 / w3 in 4-d-chunk
                    # pieces (x on the scalar queue, weights on sync) so the
                    # first partial accumulation (d0..d3 of both chains) can
                    # start after ~0.4MB of DMA.
                    C = slices[0][0]
                    x_t = slices[0][1]
                    for p in range(4):
                        xa, xb_ = p * 4 * C, (p + 1) * 4 * C
                        wa, wb = p * 4 * 128, (p + 1) * 4 * 128
                        nc.scalar.dma_start(xg[0][:, xa:xb_], x_t[:, xa:xb_])
                        nc.sync.dma_start(wt1[:, wa:wb],
                                          w1_t[:, lo + wa:lo + wb])
                        nc.sync.dma_start(wt3[:, wa:wb],
                                          w3_t[:, lo + wa:lo + wb])
                else:
                    # per-block transfers (not per-group): halves the
                    # delivery granularity so the startup compute wave never
                    # outruns delivery by a whole 2-block group.
                    for sb in range(nb):
                        blo = lo + sb * DB * 128
                        nc.sync.dma_start(
                            wt1[:, sb * DB * 128:(sb + 1) * DB * 128],
                            w1_t[:, blo:blo + DB * 128])
                        nc.sync.dma_start(
                            wt3[:, sb * DB * 128:(sb + 1) * DB * 128],
                            w3_t[:, blo:blo + DB * 128])
                if s == 0 and gi == 9:
                    # slice-1 x, issued past the startup ramp so it
                    # streams during slice-0 compute (needed much later).
                    C2_, xb = slices[1][0], slices[1][1]
                    for q0 in range(0, DB * C2_, 8 * C2_):
                        q1 = min(q0 + 8 * C2_, DB * C2_)
                        nc.sync.dma_start(xg[1][:, q0:q1], xb[:, q0:q1])
                return wt1, wt3

            def groups_of(s):
                return ([1, 1] + [2] * 10) if s == 0 else [2] * 11

            def emit_phase1(s, h, pre_issued, w2_stash=None):
                """Phase 1 matmuls + silu for slice s. pre_issued: list of
                (wt1, wt3, nb, b0) groups already DMA'd during the previous
                slice's phase 2. If w2_stash is a list, prefetch up to 10 of
                this slice's w2 groups (paced one per w13 group) into it —
                phase 2 of the LAST slice can't rely on the empty-ring
                prefetch that slice 0 gets for free."""
                C = slices[s][0]
                w2_t = slices[s][4]
                w2groups = [(ph, g0, min(GSZ, HB - g0))
                            for ph in range(NPASS) for g0 in range(0, HB, GSZ)]
                glist = []
                b0 = 0
                for gi, nb in enumerate(groups_of(s)):
                    if gi < len(pre_issued):
                        glist.append(pre_issued[gi])
                    else:
                        wt1, wt3 = issue_w13(s, gi, nb, b0)
                        glist.append((wt1, wt3, nb, b0))
                    b0 += nb
                    if w2_stash is not None and len(w2_stash) < 20:
                        for _ in range(2):
                            if len(w2_stash) >= 20:
                                break
                            ph, g0, nb2 = w2groups[len(w2_stash)]
                            wt2 = w2pool.tile([128, GSZ * W], F8E3, tag="w2",
                                              name="wt2")
                            nc.sync.dma_start(
                                wt2[:, :nb2 * W],
                                w2_t[ph][:, g0 * W:(g0 + nb2) * W])
                            w2_stash.append((wt2, g0, nb2))
                for wt1, wt3, nb, b0 in glist:
                    for sb in range(nb):
                        b = b0 + sb
                        ps1 = ps.tile([128, C1], F32, tag="ps1")
                        ps3 = ps.tile([128, C1], F32, tag="ps3")
                        if s == 0 and b == 0:
                            # startup block: alternate w1/w3 4-d-chunk
                            # sub-chains so compute begins once DMA piece 0
                            # lands (matches the interleaved DMA order).
                            for p in range(4):
                                for do in range(4 * p, 4 * p + 4):
                                    lo = (sb * DB + do) * 128
                                    nc.tensor.matmul(
                                        ps1[:, :C], wt1[:, lo:lo + 128],
                                        xg[s][:, do * C:(do + 1) * C],
                                        start=(do == 0), stop=(do == DB - 1))
                                for do in range(4 * p, 4 * p + 4):
                                    lo = (sb * DB + do) * 128
                                    nc.tensor.matmul(
                                        ps3[:, :C], wt3[:, lo:lo + 128],
                                        xg[s][:, do * C:(do + 1) * C],
                                        start=(do == 0), stop=(do == DB - 1))
                        else:
                            for do in range(DB):
                                lo = (sb * DB + do) * 128
                                nc.tensor.matmul(
                                    ps1[:, :C], wt1[:, lo:lo + 128],
                                    xg[s][:, do * C:(do + 1) * C],
                                    start=(do == 0), stop=(do == DB - 1))
                            for do in range(DB):
                                lo = (sb * DB + do) * 128
                                nc.tensor.matmul(
                                    ps3[:, :C], wt3[:, lo:lo + 128],
                                    xg[s][:, do * C:(do + 1) * C],
                                    start=(do == 0), stop=(do == DB - 1))
                        qblk = s == 0 and b < QB
                        sig = spool.tile([128, C1], F32, tag="sig")
                        # sigmoid(g1); for fp8 startup blocks PSUM holds
                        # 128*g1, descaled via the activation input scale.
                        nc.scalar.activation(
                            sig[:, :C], ps1[:, :C],
                            mybir.ActivationFunctionType.Sigmoid,
                            scale=(1.0 / QSCALE) if qblk else 1.0)
                        m1 = spool.tile([128, C1], F32, tag="m1")
                        nc.vector.tensor_mul(m1[:, :C], sig[:, :C], ps3[:, :C])
                        if qblk:
                            m2 = spool.tile([128, C1], F32, tag="m2")
                            nc.vector.tensor_mul(m2[:, :C], m1[:, :C],
                                                 ps1[:, :C])
                            nc.vector.tensor_scalar_mul(
                                h[:, b * C:(b + 1) * C], m2[:, :C],
                                1.0 / (QSCALE * QSCALE))
                        else:
                            nc.vector.tensor_mul(
                                h[:, b * C:(b + 1) * C], m1[:, :C],
                                ps1[:, :C])

            def emit_phase2(s, h, next_slice, stash=()):
                """Phase 2 for slice s. If next_slice is set, weave the next
                slice's first phase-1 weight-group DMAs between early w2
                groups (sync issues them while w2 prefetch is still in its
                immediate-fire window) and return them for emit_phase1.
                stash: w2 group tiles already DMA'd during this slice's
                phase 1 (in ph-major group order)."""
                C, _x, _w1, _w3, w2_t, y_t = slices[s]
                w2groups = [(g0, min(GSZ, HB - g0)) for g0 in range(0, HB, GSZ)]
                pre = []
                gctr = 0
                for ph in range(NPASS):
                    po = {}
                    for dc in range(NDC):
                        po[dc] = ps.tile([128, C1], F32, tag=f"y{dc}",
                                         name=f"po{dc}")
                    wts = []
                    for g0, nb in w2groups:
                        if gctr < len(stash):
                            wt2, g0s, nbs = stash[gctr]
                            assert g0s == g0 and nbs == nb
                        else:
                            wt2 = w2pool.tile([128, GSZ * W], F8E3, tag="w2",
                                              name="wt2")
                            nc.sync.dma_start(
                                wt2[:, :nb * W],
                                w2_t[ph][:, g0 * W:(g0 + nb) * W])
                        wts.append((wt2, g0, nb))
                        gctr += 1
                        if (next_slice is not None and gctr % 2 == 0
                                and len(pre) < 4):
                            gi = len(pre)
                            nb_n = groups_of(next_slice)[gi]
                            b0_n = sum(groups_of(next_slice)[:gi])
                            wt1n, wt3n = issue_w13(next_slice, gi, nb_n, b0_n)
                            pre.append((wt1n, wt3n, nb_n, b0_n))
                    # un-interleaved dc chains: dc0's drain (copy + output
                    # DMA) hides under dc1's matmul chain.
                    for dc in range(NDC):
                        for wt2, g0, nb in wts:
                            for sb in range(nb):
                                b = g0 + sb
                                lo = sb * W + dc * 128
                                nc.tensor.matmul(
                                    po[dc][:, :C],
                                    wt2[:, lo:lo + 128],
                                    h[:, b * C:(b + 1) * C],
                                    start=(b == 0), stop=(b == HB - 1))
                        ot = opool.tile([128, C1], WDT, tag="ot")
                        # fused descale of the fp8 w2 pre-scale (2**-7)
                        nc.vector.tensor_scalar_mul(
                            ot[:, :C], po[dc][:, :C], 1.0 / 128.0)
                        nc.scalar.dma_start(
                            y_t[ph * W + dc * 128:ph * W + dc * 128 + 128, :],
                            ot[:, :C])
                return pre

            h0 = hpool.tile([128, HB * C1], WDT, tag="h0")
            h1 = hpool.tile([128, HB * C2], WDT, tag="h1")
            emit_phase1(0, h0, [])
            pre_b = emit_phase2(0, h0, next_slice=1)
            stash_b = []
            emit_phase1(1, h1, pre_b, w2_stash=stash_b)
            emit_phase2(1, h1, next_slice=None, stash=stash_b)

    nc.compile()
    _BUILD_CACHE[key] = nc
    return nc


def _pack13(wh):
    """[nb*128, 2048] w1/w3 rows -> phase-1 layout [128, nb*DB*128]:
    col = (b*DB + do)*128 + i_in, partition = d_in."""
    nb = wh.shape[0] // 128
    return np.ascontiguousarray(
        wh.reshape(nb, 128, DB, 128).transpose(3, 0, 2, 1)
    ).reshape(128, nb * DB * 128)


F8E3 = ml_dtypes.float8_e3m4
QSCALE = 128.0    # power-of-2 pre-scale: keeps weights out of e3m4 subnormals
W2SCALE = QSCALE


def _pack2(wh):
    """[2816, 2048] f32 w2 half -> fp8 e3m4 phase-2 layout
    [NPASS, 128, HB*W]: per pass, col = b*W + j, partition = i_in."""
    q = (wh * W2SCALE).astype(F8E3)
    return np.ascontiguousarray(
        q.reshape(HB, 128, NPASS, W).transpose(2, 1, 0, 3)
    ).reshape(NPASS, 128, HB * W)


def _packx(x_bf, tokens, C):
    """Gather token rows of x (bf16) and lay out as [128, DB*C]:
    col = do*C + c, partition = d_in."""
    xp = np.zeros((C, D), BF16)
    xp[:len(tokens)] = x_bf[tokens]
    return np.ascontiguousarray(
        xp.reshape(C, DB, 128).transpose(2, 1, 0)
    ).reshape(128, DB * C)


def _prepare(inputs):
    """Host routing + packing. Returns (nc, in_maps, scatter_info)."""
    x = np.asarray(inputs["x"])
    idx = np.asarray(inputs["expert_indices"])
    w1 = np.asarray(inputs["w1"])
    w2 = np.asarray(inputs["w2"])
    w3 = np.asarray(inputs["w3"])

    t_n, a_n = idx.shape

    # ---- dedup + routing ----
    tt = np.repeat(np.arange(t_n), a_n)
    ee = idx.reshape(-1).astype(np.int64)
    keys = tt * E + ee
    uniq = np.unique(keys)                        # sorted (t, e) pairs
    ue = uniq % E
    ut = uniq // E
    order = np.argsort(ue, kind="stable")         # grouped by expert
    counts = np.bincount(ue, minlength=E)
    starts = np.concatenate([[0], np.cumsum(counts)])
    # concat-layout row of each unique pair, and the gather map for scatter
    col = np.empty(len(uniq), np.int64)
    col[order] = np.arange(len(uniq)) - starts[ue[order]]
    concat_row = starts[ue] + col
    gather_rows = concat_row[np.searchsorted(uniq, keys)]   # [T*A]

    # ---- heavy/light pairing ----
    rank = np.argsort(-counts, kind="stable")
    pairs = [(int(rank[i]), int(rank[7 - i])) for i in range(4)]
    C1 = _pad4(counts[rank[0]])
    C2 = _pad4(counts[rank[4]])
    tokens_of = {
        int(e): ut[order[starts[e]:starts[e] + counts[e]]] for e in range(E)
    }

    nc = _build(C1, C2)

    x_bf = x.astype(BF16)
    w1_bf = {}
    in_maps = [dict() for _ in range(N_CORES)]
    for g, (he, le) in enumerate(pairs):
        xa = _packx(x_bf, tokens_of[he], C1)
        xb = _packx(x_bf, tokens_of[le], C2)
        for half in range(2):
            c = 2 * g + half
            r0, r1 = half * (I // 2), (half + 1) * (I // 2)
            in_maps[c]["xga"] = xa
            in_maps[c]["xgb"] = xb
            wa1, wa3 = w1[he][r0:r1], w3[he][r0:r1]
            in_maps[c]["w1aq"] = _pack13(
                (wa1[:QB * 128] * QSCALE).astype(F8E3))
            in_maps[c]["w3aq"] = _pack13(
                (wa3[:QB * 128] * QSCALE).astype(F8E3))
            in_maps[c]["w1a"] = _pack13(wa1[QB * 128:].astype(BF16))
            in_maps[c]["w3a"] = _pack13(wa3[QB * 128:].astype(BF16))
            in_maps[c]["w2a"] = _pack2(w2[he][r0:r1])
            in_maps[c]["w1b"] = _pack13(w1[le][r0:r1].astype(BF16))
            in_maps[c]["w3b"] = _pack13(w3[le][r0:r1].astype(BF16))
            in_maps[c]["w2b"] = _pack2(w2[le][r0:r1])

    scatter_info = (t_n, a_n, pairs, counts, starts, gather_rows, len(uniq))
    return nc, in_maps, scatter_info


def _scatter(results, scatter_info):
    t_n, a_n, pairs, counts, starts, gather_rows, n_uniq = scatter_info
    yc = np.empty((n_uniq, D), np.float32)
    for g, (he, le) in enumerate(pairs):
        ya = (results[2 * g]["yta"].astype(np.float32)
              + results[2 * g + 1]["yta"].astype(np.float32))     # [D, C1]
        yb = (results[2 * g]["ytb"].astype(np.float32)
              + results[2 * g + 1]["ytb"].astype(np.float32))     # [D, C2]
        yc[starts[he]:starts[he] + counts[he]] = ya[:, :counts[he]].T
        yc[starts[le]:starts[le] + counts[le]] = yb[:, :counts[le]].T
    return yc[gather_rows].reshape(t_n, a_n, D)


def kernel(**inputs):
    from concourse.bass_utils import run_bass_kernel_spmd

    nc, in_maps, scatter_info = _prepare(inputs)
    res = run_bass_kernel_spmd(nc, in_maps, core_ids=list(range(N_CORES)))
    return _scatter(res.results, scatter_info)

